# revision 29
# baseline (speedup 1.0000x reference)
"""Trainium2 Bass kernel for nn_CachedAttention (8-core SPMD, tensor-parallel heads).

Contract: kernel(**inputs) takes the FULL unsharded inputs from
reference.setup_inputs() and returns the FULL (1, 2048, 2048) f32 output.

Math notes (validated against the reference):
- TOP-LEFT-aligned causal mask tril(T, S): new token t only attends cache
  positions 0..t, so the fresh k/v projections are fully masked -> skipped.
- RMSNorm commutes with RoPE (rotation preserves per-token norm); q_norm_w
  folds into the rope tables. rstd computed post-projection via
  DVE square -> ones-matmul partition reduce -> ln/exp rsqrt (one ACT
  table set: natural_log_exp covers Ln and Exp, zero table thrash).
- Scores ~ N(0,1): softmax without max-subtraction; row sum from a
  ones-column appended to V (dk+1 = 129-wide pV matmuls).

Layout/scheduling notes:
- qproj computes qT[dk, t] directly (lhsT = wqT chunks, rhs = xT windows,
  free=512): no transposes; rstd applied via selector-matmul broadcast.
- Attention head 0 groups interleave with qproj windows (group g needs
  only window g's qT); head 1 follows. exp batched over 2-tile PSUM slabs.
- Per-head AllToAll: h0's overlaps h1's attention; wo head-0 half-chains
  run under h1's AllToAll. a2a receive uses HWDGE DMA-transpose straight
  into the wo lhsT layout.
- DMA sequencing: sync(SP) HWDGE FIFO carries wq, x windows, tables, then
  the full 8MB wo prefetch (ordered behind x so qproj is never starved);
  scalar(ACT) ring carries warmup + a2a staging; sync ring later carries
  the a2a-receive transposes + output stores.
- A tiny warmup AllToAll issued at t=0 absorbs the ~55us one-time
  collective arming cost under the compute phases.
"""

import math
import sys

import numpy as np

sys.path.insert(0, "/opt/trn_rl_repo")

import ml_dtypes

P = 128
T = 2048
DM = 2048
DK = 128
HLOC = 2          # q heads per core
NCORES = 8
TW = 512          # qproj token window
NW = T // TW      # 4 windows
ND = DM // P      # 16 contraction chunks
NS = T // P       # 16 cache s-tiles
GW = 4            # token tiles per attention group (512 wide)
NG = NS // GW     # 4 groups
NTL = T // NCORES // P   # 2 local token tiles after resharding
EPS = 1e-6
ROPE_BASE = 10000.0

_bf16 = ml_dtypes.bfloat16


def _build_module():
    import concourse.tile as tile
    from concourse import bacc, mybir

    bf = mybir.dt.bfloat16
    f32 = mybir.dt.float32
    AF = mybir.ActivationFunctionType

    nc = bacc.Bacc("TRN2", target_bir_lowering=False, debug=False, num_devices=NCORES)

    xT = nc.dram_tensor("xT", [DM, T], bf, kind="ExternalInput").ap()
    wqT = nc.dram_tensor("wqT", [DM, HLOC * DK], bf, kind="ExternalInput").ap()
    cosT = nc.dram_tensor("cosT", [DK, T], bf, kind="ExternalInput").ap()
    sinT = nc.dram_tensor("sinT", [DK, T], bf, kind="ExternalInput").ap()
    kcT = nc.dram_tensor("kcT", [DK, T], bf, kind="ExternalInput").ap()
    vca = nc.dram_tensor("vca", [T, DK + 1], bf, kind="ExternalInput").ap()
    tri = nc.dram_tensor("tri", [P, P], bf, kind="ExternalInput").ap()
    woT = nc.dram_tensor("woT", [DM, DM], bf, kind="ExternalInput").ap()
    out = nc.dram_tensor("out", [T // NCORES, DM], bf, kind="ExternalOutput").ap()

    with tile.TileContext(nc) as tc:
        with (
            tc.tile_pool(name="res", bufs=1) as res,
            tc.tile_pool(name="xpool", bufs=2) as xpool,
            tc.tile_pool(name="qsqp", bufs=2) as qsqp,
            tc.tile_pool(name="work", bufs=3) as work,
            tc.tile_pool(name="lnp", bufs=2) as lnp,
            tc.tile_pool(name="pb", bufs=18) as pbp,
            tc.tile_pool(name="small", bufs=4) as small,
            tc.tile_pool(name="osb", bufs=4) as osbp,
            tc.tile_pool(name="ps_qp", bufs=2, space="PSUM") as ps_qp,
            tc.tile_pool(name="ps_ax", bufs=2, space="PSUM") as ps_ax,
            tc.tile_pool(name="ps_sc", bufs=2, space="PSUM") as ps_sc,
            tc.tile_pool(name="dram", bufs=1, space="DRAM") as dram,
        ):
            # ---- t=0: arm the collectives path (one-time ~55us, hidden).
            # Feed the warmup a2a by a DRAM->DRAM copy from an external
            # input: no engine dependency, so the trigger fires immediately.
            warm_in = dram.tile([NCORES, 16], bf, name="warm_in")
            warm_out = dram.tile([NCORES, 16], bf, name="warm_out")
            nc.sync.dma_start(warm_in, tri[0:NCORES, 0:16])
            for _ in range(2):   # 2nd tiny a2a absorbs the post-arming ramp
                nc.gpsimd.collective_compute(
                    "AllToAll",
                    mybir.AluOpType.bypass,
                    ins=[warm_in.opt()],
                    outs=[warm_out.opt()],
                    replica_groups=[list(range(NCORES))],
                )

            # ---- sync(SP)-ring FIFO loads, in consumption order ----
            wq_sb = res.tile([P, ND, HLOC * DK], bf)
            nc.sync.dma_start(wq_sb, wqT.rearrange("(d p) f -> p d f", p=P))
            xT_r = xT.rearrange("(d p) t -> p d t", p=P)
            x_sb = []
            for w in range(NW):
                xw = xpool.tile([P, ND, TW], bf, tag="x")
                nc.sync.dma_start(xw, xT_r[:, :, w * TW:(w + 1) * TW])
                x_sb.append(xw)
                if w == 0:
                    cos_sb = res.tile([P, T], bf)
                    nc.sync.dma_start(cos_sb, cosT)
                    sin_sb = res.tile([P, T], bf)
                    nc.sync.dma_start(sin_sb, sinT)
            kc_sb = res.tile([P, T], bf)
            nc.sync.dma_start(kc_sb, kcT)
            vca_sb = res.tile([P, NS, DK + 1], bf)
            nc.sync.dma_start(vca_sb, vca.rearrange("(s p) d -> p s d", p=P))
            tri_sb = res.tile([P, P], bf)
            nc.sync.dma_start(tri_sb, tri)
            # full wo prefetch, sequenced behind x on the same FIFO
            wo_sb = res.tile([P, ND, DM], bf)
            woT_r = woT.rearrange("(o p) f -> p o f", p=P)
            for o in range(4):
                nc.sync.dma_start(wo_sb[:, o * 4:(o + 1) * 4, :],
                                  woT_r[:, o * 4:(o + 1) * 4, :])

            ones_sb = res.tile([P, 1], bf)
            nc.vector.memset(ones_sb, 1.0)
            ones_row = res.tile([1, P], f32)
            nc.vector.memset(ones_row, 1.0)
            eps_sb = res.tile([1, 1], f32)
            nc.vector.memset(eps_sb, EPS)

            qT = [res.tile([P, T], bf, name=f"qT{h}") for h in range(HLOC)]
            att_sb = [res.tile([P, NS, DK], bf, name=f"att{h}")
                      for h in range(HLOC)]

            # ---- qproj window: qT[dk, t] directly; rstd via ln/exp.
            # The rstd broadcast+apply for the PREVIOUS (w,h) is emitted
            # after the current matmul chain (lag-1 pipeline) so the tensor
            # queue never stalls waiting on the ACT ln/exp chain.
            pend = [None]

            def flush_rstd():
                if pend[0] is None:
                    return
                rw, qr, h, cs = pend[0]
                rb = ps_ax.tile([P, TW], f32, tag="ax")
                nc.tensor.matmul(rb, lhsT=ones_row, rhs=rw,
                                 start=True, stop=True)
                nc.vector.tensor_mul(qT[h][:, cs], qr, rb)
                pend[0] = None

            def qproj_window(w):
                for h in range(HLOC):
                    qp = ps_qp.tile([P, TW], f32, tag="qp")
                    for d in range(ND):
                        nc.tensor.matmul(
                            qp,
                            lhsT=wq_sb[:, d, h * DK:(h + 1) * DK],
                            rhs=x_sb[w][:, d, :],
                            start=(d == 0),
                            stop=(d == ND - 1),
                        )
                    # rope: qr = qp*cos + swap_halves(qp)*sin'
                    H = DK // 2
                    u = work.tile([P, TW], bf, tag="u")
                    cs = slice(w * TW, (w + 1) * TW)
                    nc.vector.tensor_mul(u[0:H, :], qp[H:P, :], sin_sb[0:H, cs])
                    nc.vector.tensor_mul(u[H:P, :], qp[0:H, :], sin_sb[H:P, cs])
                    t1 = work.tile([P, TW], bf, tag="t1")
                    nc.vector.tensor_mul(t1, qp, cos_sb[:, cs])
                    qr = work.tile([P, TW], bf, tag="qr")
                    nc.vector.tensor_add(qr, t1, u)
                    # sum_dk q^2 from qr (rotation preserves the norm):
                    # DVE square then ones-matmul partition reduce
                    qsq = qsqp.tile([P, TW], bf, tag="qsq")
                    nc.vector.tensor_mul(qsq, qr, qr)
                    ssq = ps_ax.tile([P, TW], f32, tag="ax")
                    nc.tensor.matmul(ssq[0:1, :], lhsT=ones_sb, rhs=qsq,
                                     start=True, stop=True)
                    flush_rstd()
                    # rstd = exp(-0.5 * ln(ssq/DK + eps)); Ln+Exp share a set
                    lnb = lnp.tile([1, TW], f32, tag="ln")
                    nc.scalar.activation(out=lnb, in_=ssq[0:1, :], func=AF.Ln,
                                         bias=eps_sb, scale=1.0 / DK)
                    rw = lnp.tile([1, TW], f32, tag="rw")
                    nc.scalar.activation(out=rw, in_=lnb, func=AF.Exp,
                                         scale=-0.5)
                    pend[0] = (rw, qr, h, cs)

            # ---- attention: scores+exp and pV emitted separately so pV
            # (which waits on exp) never blocks independent tensor work ----
            def att_scores(h, g):
                t0 = g * TW
                nb = 2 * (g + 1)          # si-batches of 2
                pb_tiles = []
                for b in range(nb):
                    sc = ps_sc.tile([P, 2, TW], f32, tag="sc")
                    for k2 in range(2):
                        si = 2 * b + k2
                        nc.tensor.matmul(
                            sc[:, k2, :],
                            lhsT=kc_sb[:, si * P:(si + 1) * P],
                            rhs=qT[h][:, t0:t0 + TW],
                            start=True, stop=True,
                        )
                    pb = pbp.tile([P, 2, TW], bf, tag="pb")
                    nc.scalar.activation(out=pb, in_=sc, func=AF.Exp)
                    for k2 in range(2):
                        si = 2 * b + k2
                        k = si - g * GW
                        if 0 <= k < GW:   # diagonal tile: mask s>t block
                            nc.vector.tensor_mul(
                                pb[:, k2, k * P:(k + 1) * P],
                                pb[:, k2, k * P:(k + 1) * P], tri_sb)
                    pb_tiles.append(pb)
                return pb_tiles

            def att_pv(h, g, pb_tiles):
                for tj in range(GW):
                    ti = g * GW + tj
                    po = ps_qp.tile([P, TW], f32, tag="qp")
                    for si in range(ti + 1):
                        b, k2 = divmod(si, 2)
                        nc.tensor.matmul(
                            po[:, :DK + 1],
                            lhsT=pb_tiles[b][:, k2, tj * P:(tj + 1) * P],
                            rhs=vca_sb[:, si, :],
                            start=(si == 0), stop=(si == ti),
                        )
                    recip = small.tile([P, 1], f32, tag="recip")
                    nc.vector.reciprocal(recip, po[:, DK:DK + 1])
                    nc.vector.tensor_scalar_mul(
                        att_sb[h][:, ti, :], po[:, :DK], recip)

            # ---- qproj with head-0 attention interleaved: scores lag the
            # windows by 1, pV lags scores by 1 more ----
            pbq = {}
            for w in range(NW):
                qproj_window(w)
                if w >= 1:
                    pbq[w - 1] = att_scores(0, w - 1)
                if w >= 2:
                    att_pv(0, w - 2, pbq.pop(w - 2))
            flush_rstd()
            pbq[NG - 1] = att_scores(0, NG - 1)
            att_pv(0, NG - 2, pbq.pop(NG - 2))

            # ---- a2a head 0 (overlaps head-1 attention) ----
            a_in = [dram.tile([T, DK], bf, name=f"a_in{h}") for h in range(HLOC)]
            a_out = [dram.tile([T, DK], bf, name=f"a_out{h}") for h in range(HLOC)]

            def a2a(h):
                nc.sync.dma_start(
                    a_in[h].rearrange("(t p) d -> p t d", p=P), att_sb[h])
                nc.gpsimd.collective_compute(
                    "AllToAll",
                    mybir.AluOpType.bypass,
                    ins=[a_in[h].opt()],
                    outs=[a_out[h].opt()],
                    replica_groups=[list(range(NCORES))],
                )

            # head-1 scores feed ACT continuously while head-0 pV drains
            sc1 = {0: att_scores(1, 0)}
            att_pv(0, NG - 1, pbq.pop(NG - 1))
            sc1[1] = att_scores(1, 1)
            a2a(0)
            att_pv(1, 0, sc1.pop(0))
            sc1[2] = att_scores(1, 2)
            att_pv(1, 1, sc1.pop(1))
            sc1[3] = att_scores(1, 3)
            att_pv(1, 2, sc1.pop(2))
            att_pv(1, 3, sc1.pop(3))
            a2a(1)

            # ---- a2a receive: DMA-transpose into wo lhsT layout ----
            aoT = [res.tile([P, T], bf, name=f"aoT{h}") for h in range(HLOC)]
            for h in range(HLOC):
                nc.sync.dma_start_transpose(aoT[h], a_out[h])

            # ---- wo: 8 chains; head-0 halves run under a2a1 ----
            out_r = out.rearrange("(tj p) f -> p tj f", p=P)
            WCH = 512
            NCH = DM // WCH
            wo_ps = {}
            for nch in range(2):
                scb = ps_sc.tile([P, 2, TW], f32, tag="sc")
                for tj in range(NTL):
                    wo_ps[(nch, tj)] = scb[:, tj, :]
            for tj in range(NTL):
                wop2 = ps_qp.tile([P, TW], f32, tag="qp", name=f"wop2_{tj}")
                wo_ps[(2, tj)] = wop2
                wop3 = ps_ax.tile([P, TW], f32, tag="ax", name=f"wop3_{tj}")
                wo_ps[(3, tj)] = wop3

            def half_chain(h, nch, tj, start, stop):
                pout = wo_ps[(nch, tj)]
                for i in range(NCORES):
                    nc.tensor.matmul(
                        pout,
                        lhsT=aoT[h][:, i * 256 + tj * P:i * 256 + (tj + 1) * P],
                        rhs=wo_sb[:, 2 * i + h, nch * WCH:(nch + 1) * WCH],
                        start=(start and i == 0),
                        stop=(stop and i == NCORES - 1),
                    )

            for nch in range(NCH):
                for tj in range(NTL):
                    half_chain(0, nch, tj, True, False)
            for nch in range(NCH):
                for tj in range(NTL):
                    half_chain(1, nch, tj, False, True)
                    osb = osbp.tile([P, WCH], bf, tag="osb")
                    nc.vector.tensor_copy(osb, wo_ps[(nch, tj)])
                    nc.sync.dma_start(
                        out_r[:, tj, nch * WCH:(nch + 1) * WCH], osb)

    nc.compile()
    return nc


def _host_inputs(x, cached_k, cached_v, wq, wo, q_norm_w):
    """Build the 8 per-core input maps (host-side shard + fold + cast)."""
    xt = np.ascontiguousarray(x[0].T).astype(_bf16)           # (DM, T)
    wot = np.ascontiguousarray(wo.T).astype(_bf16)            # (DM, DM), full

    inv_freq = 1.0 / (ROPE_BASE ** (np.arange(0, DK, 2, dtype=np.float32) / DK))
    ang = np.arange(T, dtype=np.float32)[:, None] * inv_freq[None, :]
    cos_f = np.concatenate([np.cos(ang), np.cos(ang)], axis=1)  # (T, DK)
    sin_f = np.concatenate([np.sin(ang), np.sin(ang)], axis=1)
    w = q_norm_w.astype(np.float32)
    C = (w[None, :] * cos_f).astype(np.float32)
    Sp = np.empty((T, DK), np.float32)
    Sp[:, :DK // 2] = -w[None, DK // 2:] * sin_f[:, :DK // 2]
    Sp[:, DK // 2:] = w[None, :DK // 2] * sin_f[:, DK // 2:]
    cosT = np.ascontiguousarray(C.T).astype(_bf16)            # (DK, T)
    sinT = np.ascontiguousarray(Sp.T).astype(_bf16)

    tri = (np.arange(P)[:, None] <= np.arange(P)[None, :]).astype(_bf16)

    in_maps = []
    for c in range(NCORES):
        fs = slice(c * HLOC * DK, (c + 1) * HLOC * DK)
        wqT = np.ascontiguousarray(wq[fs, :].T).astype(_bf16)
        kcT = np.ascontiguousarray(cached_k[c].T / math.sqrt(DK)).astype(_bf16)
        vcaa = np.concatenate(
            [cached_v[c], np.ones((T, 1), np.float32)], axis=1).astype(_bf16)
        in_maps.append({
            "xT": xt, "wqT": wqT, "cosT": cosT, "sinT": sinT,
            "kcT": kcT, "vca": vcaa, "tri": tri, "woT": wot,
        })
    return in_maps


_CACHED = {}


def _get_module():
    if "nc" not in _CACHED:
        _CACHED["nc"] = _build_module()
    return _CACHED["nc"]


def run(inputs, trace=False, **kw):
    """Compile (cached), run on 8 cores, return (output, BassKernelResults)."""
    from concourse import bass_utils

    nc = _get_module()
    in_maps = _host_inputs(
        np.asarray(inputs["x"], np.float32),
        np.asarray(inputs["cached_k"], np.float32),
        np.asarray(inputs["cached_v"], np.float32),
        np.asarray(inputs["wq"], np.float32),
        np.asarray(inputs["wo"], np.float32),
        np.asarray(inputs["q_norm_w"], np.float32),
    )
    res = bass_utils.run_bass_kernel_spmd(
        nc, in_maps, core_ids=list(range(NCORES)), trace=trace, **kw)
    rows = [res.results[c]["out"].astype(np.float32) for c in range(NCORES)]
    full = np.concatenate(rows, axis=0).reshape(1, T, DM)
    return full, res


def kernel(**inputs):
    full, _ = run(inputs)
    return full


# revision 34
# speedup vs baseline: 1.0300x; 1.0300x over previous
"""Trainium2 Bass kernel for nn_CachedAttention (8-core SPMD, tensor-parallel heads).

Contract: kernel(**inputs) takes the FULL unsharded inputs from
reference.setup_inputs() and returns the FULL (1, 2048, 2048) f32 output.

Math notes (validated against the reference):
- TOP-LEFT-aligned causal mask tril(T, S): new token t only attends cache
  positions 0..t, so the fresh k/v projections are fully masked -> skipped.
- RMSNorm commutes with RoPE (rotation preserves per-token norm); q_norm_w
  folds into the rope tables. rstd computed post-projection via
  DVE square -> ones-matmul partition reduce -> ln/exp rsqrt (one ACT
  table set: natural_log_exp covers Ln and Exp, zero table thrash).
- Scores ~ N(0,1): softmax without max-subtraction; row sum from a
  ones-column appended to V (dk+1 = 129-wide pV matmuls).

Layout/scheduling notes:
- qproj computes qT[dk, t] directly (lhsT = wqT chunks, rhs = xT windows,
  free=512): no transposes; rstd applied via selector-matmul broadcast.
- Attention head 0 groups interleave with qproj windows (group g needs
  only window g's qT); head 1 follows. exp batched over 2-tile PSUM slabs.
- Per-head AllToAll: h0's overlaps h1's attention; wo head-0 half-chains
  run under h1's AllToAll. a2a receive uses HWDGE DMA-transpose straight
  into the wo lhsT layout.
- DMA sequencing: sync(SP) HWDGE FIFO carries wq, x windows, tables, then
  the full 8MB wo prefetch (ordered behind x so qproj is never starved);
  scalar(ACT) ring carries warmup + a2a staging; sync ring later carries
  the a2a-receive transposes + output stores.
- A tiny warmup AllToAll issued at t=0 absorbs the ~55us one-time
  collective arming cost under the compute phases.
"""

import math
import sys

import numpy as np

sys.path.insert(0, "/opt/trn_rl_repo")

import ml_dtypes

P = 128
T = 2048
DM = 2048
DK = 128
HLOC = 2          # q heads per core
NCORES = 8
TW = 512          # qproj token window
NW = T // TW      # 4 windows
ND = DM // P      # 16 contraction chunks
NS = T // P       # 16 cache s-tiles
GW = 4            # token tiles per attention group (512 wide)
NG = NS // GW     # 4 groups
NTL = T // NCORES // P   # 2 local token tiles after resharding
EPS = 1e-6
ROPE_BASE = 10000.0

_bf16 = ml_dtypes.bfloat16


def _build_module():
    import concourse.tile as tile
    from concourse import bacc, mybir

    bf = mybir.dt.bfloat16
    f32 = mybir.dt.float32
    AF = mybir.ActivationFunctionType

    nc = bacc.Bacc("TRN2", target_bir_lowering=False, debug=False, num_devices=NCORES)

    xT = nc.dram_tensor("xT", [DM, T], bf, kind="ExternalInput").ap()
    wqT = nc.dram_tensor("wqT", [DM, HLOC * DK], bf, kind="ExternalInput").ap()
    cosT = nc.dram_tensor("cosT", [DK, T], bf, kind="ExternalInput").ap()
    sinT = nc.dram_tensor("sinT", [DK, T], bf, kind="ExternalInput").ap()
    kcT = nc.dram_tensor("kcT", [DK, T], bf, kind="ExternalInput").ap()
    vca = nc.dram_tensor("vca", [T, DK + 1], bf, kind="ExternalInput").ap()
    tri = nc.dram_tensor("tri", [P, P], bf, kind="ExternalInput").ap()
    woT = nc.dram_tensor("woT", [DM, DM], bf, kind="ExternalInput").ap()
    out = nc.dram_tensor("out", [T // NCORES, DM], bf, kind="ExternalOutput").ap()

    with tile.TileContext(nc) as tc:
        with (
            tc.tile_pool(name="res", bufs=1) as res,
            tc.tile_pool(name="xpool", bufs=2) as xpool,
            tc.tile_pool(name="qsqp", bufs=2) as qsqp,
            tc.tile_pool(name="work", bufs=3) as work,
            tc.tile_pool(name="lnp", bufs=2) as lnp,
            tc.tile_pool(name="pb", bufs=18) as pbp,
            tc.tile_pool(name="small", bufs=4) as small,
            tc.tile_pool(name="osb", bufs=4) as osbp,
            tc.tile_pool(name="ps_qp", bufs=2, space="PSUM") as ps_qp,
            tc.tile_pool(name="ps_ax", bufs=2, space="PSUM") as ps_ax,
            tc.tile_pool(name="ps_sc", bufs=2, space="PSUM") as ps_sc,
            tc.tile_pool(name="dram", bufs=1, space="DRAM") as dram,
        ):
            # ---- t=0: arm the collectives path (one-time ~55us, hidden).
            # Feed the warmup a2a by a DRAM->DRAM copy from an external
            # input: no engine dependency, so the trigger fires immediately.
            warm_in = dram.tile([NCORES, 16], bf, name="warm_in")
            warm_out = dram.tile([NCORES, 16], bf, name="warm_out")
            nc.sync.dma_start(warm_in, tri[0:NCORES, 0:16])
            for _ in range(2):   # 2nd tiny a2a absorbs the post-arming ramp
                nc.gpsimd.collective_compute(
                    "AllToAll",
                    mybir.AluOpType.bypass,
                    ins=[warm_in.opt()],
                    outs=[warm_out.opt()],
                    replica_groups=[list(range(NCORES))],
                )

            # ---- sync(SP)-ring FIFO loads, in consumption order ----
            wq_sb = res.tile([P, ND, HLOC * DK], bf)
            nc.sync.dma_start(wq_sb, wqT.rearrange("(d p) f -> p d f", p=P))
            xT_r = xT.rearrange("(d p) t -> p d t", p=P)
            x_sb = []
            for w in range(NW):
                xw = xpool.tile([P, ND, TW], bf, tag="x")
                nc.sync.dma_start(xw, xT_r[:, :, w * TW:(w + 1) * TW])
                x_sb.append(xw)
                if w == 0:
                    cos_sb = res.tile([P, T], bf)
                    nc.sync.dma_start(cos_sb, cosT)
                    sin_sb = res.tile([P, T], bf)
                    nc.sync.dma_start(sin_sb, sinT)
            kc_sb = res.tile([P, T], bf)
            nc.sync.dma_start(kc_sb, kcT)
            vca_sb = res.tile([P, NS, DK + 1], bf)
            nc.sync.dma_start(vca_sb, vca.rearrange("(s p) d -> p s d", p=P))
            tri_sb = res.tile([P, P], bf)
            nc.sync.dma_start(tri_sb, tri)
            # full wo prefetch, sequenced behind x on the same FIFO
            wo_sb = res.tile([P, ND, DM], bf)
            woT_r = woT.rearrange("(o p) f -> p o f", p=P)
            for o in range(4):
                nc.sync.dma_start(wo_sb[:, o * 4:(o + 1) * 4, :],
                                  woT_r[:, o * 4:(o + 1) * 4, :])

            ones_sb = res.tile([P, 1], bf)
            nc.vector.memset(ones_sb, 1.0)
            ones_row = res.tile([1, P], f32)
            nc.vector.memset(ones_row, 1.0)
            eps_sb = res.tile([1, 1], f32)
            nc.vector.memset(eps_sb, EPS)

            qT = [res.tile([P, T], bf, name=f"qT{h}") for h in range(HLOC)]
            att_sb = [res.tile([P, NS, DK], bf, name=f"att{h}")
                      for h in range(HLOC)]

            # ---- qproj window: qT[dk, t] directly; rstd via ln/exp.
            # The rstd broadcast+apply for the PREVIOUS (w,h) is emitted
            # after the current matmul chain (lag-1 pipeline) so the tensor
            # queue never stalls waiting on the ACT ln/exp chain.
            pend = [None]

            def flush_rstd():
                if pend[0] is None:
                    return
                rw, qr, h, cs = pend[0]
                rb = ps_ax.tile([P, TW], f32, tag="ax")
                nc.tensor.matmul(rb, lhsT=ones_row, rhs=rw,
                                 start=True, stop=True)
                nc.vector.tensor_mul(qT[h][:, cs], qr, rb)
                pend[0] = None

            def qproj_window(w):
                for h in range(HLOC):
                    qp = ps_qp.tile([P, TW], f32, tag="qp")
                    for d in range(ND):
                        nc.tensor.matmul(
                            qp,
                            lhsT=wq_sb[:, d, h * DK:(h + 1) * DK],
                            rhs=x_sb[w][:, d, :],
                            start=(d == 0),
                            stop=(d == ND - 1),
                        )
                    # rope: qr = qp*cos + swap_halves(qp)*sin'
                    H = DK // 2
                    u = work.tile([P, TW], bf, tag="u")
                    cs = slice(w * TW, (w + 1) * TW)
                    nc.vector.tensor_mul(u[0:H, :], qp[H:P, :], sin_sb[0:H, cs])
                    nc.vector.tensor_mul(u[H:P, :], qp[0:H, :], sin_sb[H:P, cs])
                    t1 = work.tile([P, TW], bf, tag="t1")
                    nc.vector.tensor_mul(t1, qp, cos_sb[:, cs])
                    qr = work.tile([P, TW], bf, tag="qr")
                    nc.vector.tensor_add(qr, t1, u)
                    # sum_dk q^2 from qr (rotation preserves the norm):
                    # DVE square then ones-matmul partition reduce
                    qsq = qsqp.tile([P, TW], bf, tag="qsq")
                    nc.vector.tensor_mul(qsq, qr, qr)
                    ssq = ps_ax.tile([P, TW], f32, tag="ax")
                    nc.tensor.matmul(ssq[0:1, :], lhsT=ones_sb, rhs=qsq,
                                     start=True, stop=True)
                    flush_rstd()
                    # rstd = exp(-0.5 * ln(ssq/DK + eps)); Ln+Exp share a set
                    lnb = lnp.tile([1, TW], f32, tag="ln")
                    nc.scalar.activation(out=lnb, in_=ssq[0:1, :], func=AF.Ln,
                                         bias=eps_sb, scale=1.0 / DK)
                    rw = lnp.tile([1, TW], f32, tag="rw")
                    nc.scalar.activation(out=rw, in_=lnb, func=AF.Exp,
                                         scale=-0.5)
                    pend[0] = (rw, qr, h, cs)

            a_in = [dram.tile([T, DK], bf, name=f"a_in{h}") for h in range(HLOC)]
            a_out = [dram.tile([T, DK], bf, name=f"a_out{h}")
                     for h in range(HLOC)]
            a_in_r = [a_in[h].rearrange("(t p) d -> p t d", p=P)
                      for h in range(HLOC)]

            def stage_a_in(h, g):
                # stage this group's att rows right after its pV completes,
                # so the collective trigger isn't gated by one big DMA
                nc.sync.dma_start(
                    a_in_r[h][:, g * GW:(g + 1) * GW, :],
                    att_sb[h][:, g * GW:(g + 1) * GW, :])

            # ---- attention: scores+exp and pV emitted separately so pV
            # (which waits on exp) never blocks independent tensor work ----
            def att_scores(h, g):
                t0 = g * TW
                nb = 2 * (g + 1)          # si-batches of 2
                pb_tiles = []
                for b in range(nb):
                    sc = ps_sc.tile([P, 2, TW], f32, tag="sc")
                    for k2 in range(2):
                        si = 2 * b + k2
                        nc.tensor.matmul(
                            sc[:, k2, :],
                            lhsT=kc_sb[:, si * P:(si + 1) * P],
                            rhs=qT[h][:, t0:t0 + TW],
                            start=True, stop=True,
                        )
                    pb = pbp.tile([P, 2, TW], bf, tag="pb")
                    nc.scalar.activation(out=pb, in_=sc, func=AF.Exp)
                    for k2 in range(2):
                        si = 2 * b + k2
                        k = si - g * GW
                        if 0 <= k < GW:   # diagonal tile: mask s>t block
                            nc.vector.tensor_mul(
                                pb[:, k2, k * P:(k + 1) * P],
                                pb[:, k2, k * P:(k + 1) * P], tri_sb)
                    pb_tiles.append(pb)
                return pb_tiles

            def att_pv(h, g, pb_tiles):
                for tj in range(GW):
                    ti = g * GW + tj
                    po = ps_qp.tile([P, TW], f32, tag="qp")
                    for si in range(ti + 1):
                        b, k2 = divmod(si, 2)
                        nc.tensor.matmul(
                            po[:, :DK + 1],
                            lhsT=pb_tiles[b][:, k2, tj * P:(tj + 1) * P],
                            rhs=vca_sb[:, si, :],
                            start=(si == 0), stop=(si == ti),
                        )
                    recip = small.tile([P, 1], f32, tag="recip")
                    nc.vector.reciprocal(recip, po[:, DK:DK + 1])
                    nc.vector.tensor_scalar_mul(
                        att_sb[h][:, ti, :], po[:, :DK], recip)
                stage_a_in(h, g)

            # ---- qproj with head-0 attention interleaved: scores lag the
            # windows by 1, pV lags scores by 1 more ----
            pbq = {}
            for w in range(NW):
                qproj_window(w)
                if w >= 1:
                    pbq[w - 1] = att_scores(0, w - 1)
                if w >= 2:
                    att_pv(0, w - 2, pbq.pop(w - 2))
            flush_rstd()
            pbq[NG - 1] = att_scores(0, NG - 1)
            att_pv(0, NG - 2, pbq.pop(NG - 2))

            # ---- a2a head 0 (overlaps head-1 attention) ----
            def a2a(h):
                nc.gpsimd.collective_compute(
                    "AllToAll",
                    mybir.AluOpType.bypass,
                    ins=[a_in[h].opt()],
                    outs=[a_out[h].opt()],
                    replica_groups=[list(range(NCORES))],
                )

            # head-1 scores feed ACT continuously while head-0 pV drains
            sc1 = {0: att_scores(1, 0)}
            att_pv(0, NG - 1, pbq.pop(NG - 1))
            sc1[1] = att_scores(1, 1)
            a2a(0)
            att_pv(1, 0, sc1.pop(0))
            sc1[2] = att_scores(1, 2)
            att_pv(1, 1, sc1.pop(1))
            sc1[3] = att_scores(1, 3)
            att_pv(1, 2, sc1.pop(2))
            att_pv(1, 3, sc1.pop(3))
            a2a(1)

            # ---- a2a receive: DMA-transpose into wo lhsT layout ----
            aoT = [res.tile([P, T], bf, name=f"aoT{h}") for h in range(HLOC)]
            for h in range(HLOC):
                nc.sync.dma_start_transpose(aoT[h], a_out[h])

            # ---- wo: 8 chains; head-0 halves run under a2a1 ----
            out_r = out.rearrange("(tj p) f -> p tj f", p=P)
            WCH = 512
            NCH = DM // WCH
            wo_ps = {}
            for nch in range(2):
                scb = ps_sc.tile([P, 2, TW], f32, tag="sc")
                for tj in range(NTL):
                    wo_ps[(nch, tj)] = scb[:, tj, :]
            for tj in range(NTL):
                wop2 = ps_qp.tile([P, TW], f32, tag="qp", name=f"wop2_{tj}")
                wo_ps[(2, tj)] = wop2
                wop3 = ps_ax.tile([P, TW], f32, tag="ax", name=f"wop3_{tj}")
                wo_ps[(3, tj)] = wop3

            def half_chain(h, nch, tj, start, stop):
                pout = wo_ps[(nch, tj)]
                for i in range(NCORES):
                    nc.tensor.matmul(
                        pout,
                        lhsT=aoT[h][:, i * 256 + tj * P:i * 256 + (tj + 1) * P],
                        rhs=wo_sb[:, 2 * i + h, nch * WCH:(nch + 1) * WCH],
                        start=(start and i == 0),
                        stop=(stop and i == NCORES - 1),
                    )

            for nch in range(NCH):
                for tj in range(NTL):
                    half_chain(0, nch, tj, True, False)
            for nch in range(NCH):
                for tj in range(NTL):
                    half_chain(1, nch, tj, False, True)
                    osb = osbp.tile([P, WCH], bf, tag="osb")
                    nc.vector.tensor_copy(osb, wo_ps[(nch, tj)])
                    nc.sync.dma_start(
                        out_r[:, tj, nch * WCH:(nch + 1) * WCH], osb)

    nc.compile()
    return nc


def _host_inputs(x, cached_k, cached_v, wq, wo, q_norm_w):
    """Build the 8 per-core input maps (host-side shard + fold + cast)."""
    xt = np.ascontiguousarray(x[0].T).astype(_bf16)           # (DM, T)
    wot = np.ascontiguousarray(wo.T).astype(_bf16)            # (DM, DM), full

    inv_freq = 1.0 / (ROPE_BASE ** (np.arange(0, DK, 2, dtype=np.float32) / DK))
    ang = np.arange(T, dtype=np.float32)[:, None] * inv_freq[None, :]
    cos_f = np.concatenate([np.cos(ang), np.cos(ang)], axis=1)  # (T, DK)
    sin_f = np.concatenate([np.sin(ang), np.sin(ang)], axis=1)
    w = q_norm_w.astype(np.float32)
    C = (w[None, :] * cos_f).astype(np.float32)
    Sp = np.empty((T, DK), np.float32)
    Sp[:, :DK // 2] = -w[None, DK // 2:] * sin_f[:, :DK // 2]
    Sp[:, DK // 2:] = w[None, :DK // 2] * sin_f[:, DK // 2:]
    cosT = np.ascontiguousarray(C.T).astype(_bf16)            # (DK, T)
    sinT = np.ascontiguousarray(Sp.T).astype(_bf16)

    tri = (np.arange(P)[:, None] <= np.arange(P)[None, :]).astype(_bf16)

    in_maps = []
    for c in range(NCORES):
        fs = slice(c * HLOC * DK, (c + 1) * HLOC * DK)
        wqT = np.ascontiguousarray(wq[fs, :].T).astype(_bf16)
        kcT = np.ascontiguousarray(cached_k[c].T / math.sqrt(DK)).astype(_bf16)
        vcaa = np.concatenate(
            [cached_v[c], np.ones((T, 1), np.float32)], axis=1).astype(_bf16)
        in_maps.append({
            "xT": xt, "wqT": wqT, "cosT": cosT, "sinT": sinT,
            "kcT": kcT, "vca": vcaa, "tri": tri, "woT": wot,
        })
    return in_maps


_CACHED = {}


def _get_module():
    if "nc" not in _CACHED:
        _CACHED["nc"] = _build_module()
    return _CACHED["nc"]


def run(inputs, trace=False, **kw):
    """Compile (cached), run on 8 cores, return (output, BassKernelResults)."""
    from concourse import bass_utils

    nc = _get_module()
    in_maps = _host_inputs(
        np.asarray(inputs["x"], np.float32),
        np.asarray(inputs["cached_k"], np.float32),
        np.asarray(inputs["cached_v"], np.float32),
        np.asarray(inputs["wq"], np.float32),
        np.asarray(inputs["wo"], np.float32),
        np.asarray(inputs["q_norm_w"], np.float32),
    )
    res = bass_utils.run_bass_kernel_spmd(
        nc, in_maps, core_ids=list(range(NCORES)), trace=trace, **kw)
    rows = [res.results[c]["out"].astype(np.float32) for c in range(NCORES)]
    full = np.concatenate(rows, axis=0).reshape(1, T, DM)
    return full, res


def kernel(**inputs):
    full, _ = run(inputs)
    return full
